# revision 5
# baseline (speedup 1.0000x reference)
"""LSTM cell (4-gate) Trainium2 Bass kernel, data-parallel over batch on 8 cores.

Computation (per reference):
    ih = concat(i, h, axis=1)                 # [B, K], K = 4096
    o_g = act_g(ih @ Wg.T + bg)               # gates, act = sigmoid/sigmoid/tanh/sigmoid
    new_c = c*o1 + o2*o3
    new_h = tanh(c) * o4

Strategy: shard batch B=8192 across 8 cores (1024 rows each); weights replicated.
All matmuls run in the transposed domain: out[j, b] = sum_k W_g[j, k] * ihT[k, b],
with the weight tile as the stationary operand [128k x 128j] and ihT as the moving
operand [128k x 512b]. This makes the gate bias a per-partition vector (fused into
the ScalarE activation) and keeps every DMA contiguous. Matmul inputs are bf16
(full-rate PE streaming), accumulation is fp32 in PSUM, epilogue is fp32.
"""

import numpy as np
import ml_dtypes

import concourse.bass as bass
import concourse.bacc as bacc
import concourse.mybir as mybir
from concourse.tile import TileContext
from concourse.bass_utils import run_bass_kernel_spmd

NCORES = 8
B, IN, OUT = 8192, 2048, 2048
K = IN + OUT                    # 4096 contraction dim
BLOC = B // NCORES              # 1024 batch rows per core
KT = K // 128                   # 32 k-tiles
JT = OUT // 128                 # 16 output-dim tiles per gate
NBH = BLOC // 512               # 2 batch chunks of 512

F32 = mybir.dt.float32
BF16 = mybir.dt.bfloat16
NPBF16 = ml_dtypes.bfloat16


def _reploop(reps):
    for _ in range(reps):
        yield from range(JT)


def _build(reps=1):
    nc = bacc.Bacc("TRN2", target_bir_lowering=False, debug=False, num_devices=NCORES)
    w = nc.declare_dram_parameter("w", [JT, 128, 4 * KT * 128], BF16, isOutput=False)
    ih = nc.declare_dram_parameter("ihT", [K, BLOC], BF16, isOutput=False)
    ct = nc.declare_dram_parameter("cT", [OUT, BLOC], F32, isOutput=False)
    bias = nc.declare_dram_parameter("bias", [128, 4 * JT], F32, isOutput=False)
    hT = nc.declare_dram_parameter("hT", [OUT, BLOC], F32, isOutput=True)
    cTo = nc.declare_dram_parameter("cTo", [OUT, BLOC], F32, isOutput=True)

    SIG = mybir.ActivationFunctionType.Sigmoid
    TANH = mybir.ActivationFunctionType.Tanh

    with TileContext(nc) as tc:
        with (
            tc.tile_pool(name="ihp", bufs=1) as ihp,
            tc.tile_pool(name="wp", bufs=2) as wp,
            tc.tile_pool(name="bp", bufs=1) as bp,
            tc.tile_pool(name="cp", bufs=3) as cp,
            tc.tile_pool(name="op", bufs=2) as op,
            tc.tile_pool(name="ep", bufs=3) as ep,
            tc.tile_pool(name="ps", bufs=8, space="PSUM") as psp,
        ):
            bias_t = bp.tile([128, 4 * JT], F32)
            nc.sync.dma_start(out=bias_t, in_=bias[:, :])

            # Whole ihT resident in SBUF (8 MiB bf16), one contiguous DMA per k-tile.
            ih_tiles = []
            for k in range(KT):
                t = ihp.tile([128, BLOC], BF16, tag=f"ih{k}")
                nc.sync.dma_start(out=t, in_=ih[k * 128:(k + 1) * 128, :])
                ih_tiles.append(t)

            for jt in _reploop(reps):
                # All 4 gates' weights for this 128-wide output tile: one 4 MiB
                # contiguous DMA (host pre-packs [128 part, (g,k,j)]).
                wt = wp.tile([128, 4 * KT * 128], BF16, tag="w")
                nc.sync.dma_start(out=wt, in_=w[jt])
                jsl = slice(jt * 128, (jt + 1) * 128)
                for bh in range(NBH):
                    bsl = slice(bh * 512, (bh + 1) * 512)
                    ctile = cp.tile([128, 512], F32, tag="c")
                    nc.sync.dma_start(out=ctile, in_=ct[jsl, bsl])
                    gates = []
                    for g in range(4):
                        ps = psp.tile([128, 512], F32, tag="ps")
                        for k in range(KT):
                            col = (g * KT + k) * 128
                            nc.tensor.matmul(
                                ps,
                                lhsT=wt[:, col:col + 128],
                                rhs=ih_tiles[k][:, bsl],
                                start=(k == 0),
                                stop=(k == KT - 1),
                            )
                        o = op.tile([128, 512], F32, tag=f"o{g}")
                        nc.scalar.activation(
                            o, ps, TANH if g == 2 else SIG,
                            bias=bias_t[:, jt * 4 + g: jt * 4 + g + 1],
                        )
                        gates.append(o)
                    tanhc = op.tile([128, 512], F32, tag="tanhc")
                    nc.scalar.activation(tanhc, ctile, TANH)
                    t1 = ep.tile([128, 512], F32, tag="t1")
                    nc.vector.tensor_mul(t1, ctile, gates[0])
                    t2 = ep.tile([128, 512], F32, tag="t2")
                    nc.vector.tensor_mul(t2, gates[1], gates[2])
                    nct = ep.tile([128, 512], F32, tag="nct")
                    nc.vector.tensor_add(nct, t1, t2)
                    nht = ep.tile([128, 512], F32, tag="nht")
                    nc.vector.tensor_mul(nht, tanhc, gates[3])
                    nc.sync.dma_start(out=cTo[jsl, bsl], in_=nct)
                    nc.sync.dma_start(out=hT[jsl, bsl], in_=nht)
    nc.compile()
    return nc


def _prep_inputs(i, h, c, W1, b1, W2, b2, W3, b3, W4, b4):
    ih = np.concatenate([np.asarray(i, np.float32), np.asarray(h, np.float32)], axis=1)
    W4s = np.stack([np.asarray(W1), np.asarray(W2), np.asarray(W3), np.asarray(W4)])
    # wpack[jt, p, g, k, j] = W_g[jt*128 + j, k*128 + p]
    wpack = np.ascontiguousarray(
        W4s.reshape(4, JT, 128, KT, 128).transpose(1, 4, 0, 3, 2)
    ).astype(NPBF16).reshape(JT, 128, 4 * KT * 128)
    b4s = np.stack([np.asarray(b1), np.asarray(b2), np.asarray(b3), np.asarray(b4)])
    # biaspack[p, jt*4 + g] = b_g[jt*128 + p]
    biaspack = np.ascontiguousarray(
        b4s.reshape(4, JT, 128).transpose(2, 1, 0).reshape(128, JT * 4)
    ).astype(np.float32)
    c = np.asarray(c, np.float32)

    in_maps = []
    for cs in range(NCORES):
        rows = slice(cs * BLOC, (cs + 1) * BLOC)
        ihT = np.ascontiguousarray(ih[rows].T.astype(NPBF16))
        cT = np.ascontiguousarray(c[rows].T)
        in_maps.append({"w": wpack, "ihT": ihT, "cT": cT, "bias": biaspack_f(biaspack)})
    return in_maps


def biaspack_f(b):
    return b


def _post(results):
    hT = np.concatenate([results[cs]["hT"] for cs in range(NCORES)], axis=1)
    cTo = np.concatenate([results[cs]["cTo"] for cs in range(NCORES)], axis=1)
    new_h = np.ascontiguousarray(hT.T)
    new_c = np.ascontiguousarray(cTo.T)
    return new_h, new_c


def run_full(i, h, c, W1, b1, W2, b2, W3, b3, W4, b4, trace=False, **trace_kw):
    in_maps = _prep_inputs(i, h, c, W1, b1, W2, b2, W3, b3, W4, b4)
    nc = _build()
    r = run_bass_kernel_spmd(nc, in_maps, list(range(NCORES)), trace=trace, **trace_kw)
    return _post(r.results), r


def kernel(i, h, c, W1, b1, W2, b2, W3, b3, W4, b4):
    out, _ = run_full(i, h, c, W1, b1, W2, b2, W3, b3, W4, b4, trace=False)
    return out

